# revision 2
# baseline (speedup 1.0000x reference)
"""Trainium2 Bass kernel for nn_BertWordPair (ragged RoPE pair scores).

Strategy
--------
Inputs: qw, kw (B=8, S=768, H=4, D=256) fp32; token_index, thread_id (S,) int32.
Output: (B, S, S, H) fp32 where each (row-block, col-block) pair of the 6x128
thread-block grid uses one of three RoPE sign regimes:
    pp: rope(q,+pos) . rope(k,+pos)
    np: rope(q,-pos) . rope(k,+pos)   (0 < ti_r < ti_c)
    pn: rope(q,+pos) . rope(k,-pos)   (ti_c > 0, ti_r > ti_c)

Host side precomputes the four rotated variants (q+, q-, k+, k-) in a
de-interleaved (pair-index, token) layout, casts to fp16, and shards batch
across the 8 cores (1 dialogue per core). Device does only matmuls
(one 128x128x256 contraction per output block/head, fp16 in, fp32 PSUM),
PSUM->SBUF head-interleave copies (DVE/ACT alternating), and DMAs.
"""

import os

import numpy as np

ROPE_BASE = 10000.0
B, S, H, D = 8, 768, 4, 256
HALF = D // 2  # 128
BLK = 128
NB = S // BLK  # 6
N_CORES = 8

_prog_cache = {}


def _host_rotations(qw, kw, token_index):
    """Return u/v (even/odd) rotated variants, fp32.

    Shapes: (B, S, H, HALF) each for (qp_u, qp_v, qn_u, qn_v, kp_u, kp_v,
    kn_u, kn_v)."""
    inv_freq = np.power(
        np.float32(ROPE_BASE),
        (np.arange(HALF, dtype=np.float32) * np.float32(-2.0 / D)),
    )  # (HALF,)
    pos = token_index.astype(np.float32)  # (S,)
    theta = pos[:, None] * inv_freq[None, :]  # (S, HALF)
    cos = np.cos(theta)[None, :, None, :]  # (1,S,1,HALF)
    sin = np.sin(theta)[None, :, None, :]

    out = []
    for x in (qw, kw):
        u = x[..., 0::2]  # (B,S,H,HALF)
        v = x[..., 1::2]
        uc = u * cos
        vs = v * sin
        vc = v * cos
        us = u * sin
        # positive rotation
        out.append((uc - vs, vc + us))
        # negative rotation (sin -> -sin)
        out.append((uc + vs, vc - us))
    return out  # [(qp_u,qp_v),(qn_u,qn_v),(kp_u,kp_v),(kn_u,kn_v)]


def _to_device_layout(u, v, blocks):
    """(B,S,H,HALF) u/v -> (B, H, 2, HALF, T) fp16 for the given token blocks."""
    cols = np.concatenate([np.arange(b * BLK, (b + 1) * BLK) for b in blocks])
    u = u[:, cols]  # (B,T,H,HALF)
    v = v[:, cols]
    arr = np.stack([u, v], axis=2)  # (B,T,2,H,HALF)
    arr = np.transpose(arr, (0, 3, 2, 4, 1))  # (B,H,2,HALF,T)
    return np.ascontiguousarray(arr.astype(np.float16))


def _regime_map(thread_id):
    """Return (regimes, ok). regimes[i][j] in {'pp','np','pn'} per 128-block."""
    tid = np.asarray(thread_id)
    if tid.shape[0] != S:
        return None, False
    blocks = tid.reshape(NB, BLK)
    if not np.all(blocks == blocks[:, :1]):
        return None, False  # thread blocks not aligned to 128 grid
    tvals = blocks[:, 0]
    regimes = []
    for i in range(NB):
        row = []
        for j in range(NB):
            ti_r, ti_c = tvals[i], tvals[j]
            if ti_r > 0 and ti_r < ti_c:
                row.append("np")
            elif ti_c > 0 and ti_r > ti_c:
                row.append("pn")
            else:
                row.append("pp")
        regimes.append(row)
    return regimes, True


def _build_program(regimes, qn_blocks, kn_blocks):
    import concourse.bass as bass  # noqa: F401
    import concourse.tile as tile
    from concourse import bacc, mybir

    f16 = mybir.dt.float16
    f32 = mybir.dt.float32

    nqn = max(1, len(qn_blocks))
    nkn = max(1, len(kn_blocks))
    qn_pos = {b: idx for idx, b in enumerate(qn_blocks)}
    kn_pos = {b: idx for idx, b in enumerate(kn_blocks)}

    nc = bacc.Bacc(None, target_bir_lowering=False)
    qp_d = nc.dram_tensor("qp", [H, 2, HALF, S], f16, kind="ExternalInput")
    qn_d = nc.dram_tensor("qn", [H, 2, HALF, nqn * BLK], f16, kind="ExternalInput")
    kp_d = nc.dram_tensor("kp", [H, 2, HALF, S], f16, kind="ExternalInput")
    kn_d = nc.dram_tensor("kn", [H, 2, HALF, nkn * BLK], f16, kind="ExternalInput")
    out_d = nc.dram_tensor("out", [S, S, H], f32, kind="ExternalOutput")

    with tile.TileContext(nc) as tc:
        with (
            tc.tile_pool(name="inp", bufs=1) as inp,
            tc.tile_pool(name="psum", bufs=8, space="PSUM") as pp,
            tc.tile_pool(name="stage", bufs=3) as stp,
        ):
            # Load all inputs. Tiles are (128 partitions = pair index,
            # H*2*T tokens) fp16.
            qp_t = inp.tile([HALF, H * 2 * S], f16, tag="qp")
            qn_t = inp.tile([HALF, H * 2 * nqn * BLK], f16, tag="qn")
            kp_t = inp.tile([HALF, H * 2 * S], f16, tag="kp")
            kn_t = inp.tile([HALF, H * 2 * nkn * BLK], f16, tag="kn")
            for t, d, tlen in (
                (qp_t, qp_d, S),
                (qn_t, qn_d, nqn * BLK),
                (kp_t, kp_d, S),
                (kn_t, kn_d, nkn * BLK),
            ):
                for h in range(H):
                    for c in range(2):
                        nc.sync.dma_start(
                            t[:, (h * 2 + c) * tlen : (h * 2 + c) * tlen + tlen],
                            d[h, c],
                        )

            def lhs_slice(variant, h, c, blk):
                if variant == "p":
                    return qp_t[:, (h * 2 + c) * S + blk * BLK :][:, :BLK]
                return qn_t[:, (h * 2 + c) * (nqn * BLK) + qn_pos[blk] * BLK :][:, :BLK]

            def rhs_slice(variant, h, c, blk):
                if variant == "p":
                    return kp_t[:, (h * 2 + c) * S + blk * BLK :][:, :BLK]
                return kn_t[:, (h * 2 + c) * (nkn * BLK) + kn_pos[blk] * BLK :][:, :BLK]

            copy_parity = 0
            for i in range(NB):
                stage = stp.tile([BLK, S * H], f32, tag="stage")
                for j in range(NB):
                    reg = regimes[i][j]
                    qv = "n" if reg == "np" else "p"
                    kv = "n" if reg == "pn" else "p"
                    # interleaved destination for block j: stage columns
                    # j*512 + n*4 + h
                    dst_blk = stage[:, j * (BLK * H) : (j + 1) * (BLK * H)]
                    dst_blk = dst_blk.rearrange("p (n h) -> p h n", h=H)
                    for h in range(H):
                        bank = pp.tile([BLK, BLK], f32, tag="bank")
                        for c in range(2):
                            nc.tensor.matmul(
                                bank[:],
                                lhs_slice(qv, h, c, i),
                                rhs_slice(kv, h, c, j),
                                start=(c == 0),
                                stop=(c == 1),
                            )
                        eng = nc.vector if copy_parity == 0 else nc.scalar
                        copy_parity ^= 1
                        if eng is nc.vector:
                            eng.tensor_copy(dst_blk[:, h], bank[:])
                        else:
                            eng.copy(dst_blk[:, h], bank[:])
                nc.sync.dma_start(
                    out_d[i * BLK : (i + 1) * BLK].rearrange("p n h -> p (n h)"),
                    stage[:],
                )
    nc.finalize()
    return nc


def _reference_fallback(qw, kw, token_index, thread_id):
    """Pure numpy fallback for unexpected block structure."""
    rots = _host_rotations(qw, kw, token_index)
    (qp_u, qp_v), (qn_u, qn_v), (kp_u, kp_v), (kn_u, kn_v) = rots

    def interleave(u, v):
        x = np.empty(u.shape[:-1] + (D,), dtype=np.float32)
        x[..., 0::2] = u
        x[..., 1::2] = v
        return x

    q_p = interleave(qp_u, qp_v)
    q_n = interleave(qn_u, qn_v)
    k_p = interleave(kp_u, kp_v)
    k_n = interleave(kn_u, kn_v)
    s_pp = np.einsum("bmhd,bnhd->bmnh", q_p, k_p)
    s_np = np.einsum("bmhd,bnhd->bmnh", q_n, k_p)
    s_pn = np.einsum("bmhd,bnhd->bmnh", q_p, k_n)
    ti_r = thread_id[:, None]
    ti_c = thread_id[None, :]
    sx = ((ti_r > 0) & (ti_r < ti_c))[None, :, :, None]
    sy = ((ti_c > 0) & (ti_r > ti_c))[None, :, :, None]
    return np.where(sx, s_np, np.where(sy, s_pn, s_pp)).astype(np.float32)


def kernel(qw, kw, token_index, thread_id):
    qw = np.asarray(qw, dtype=np.float32)
    kw = np.asarray(kw, dtype=np.float32)
    token_index = np.asarray(token_index)
    thread_id = np.asarray(thread_id)

    regimes, ok = _regime_map(thread_id)
    if not ok or qw.shape != (B, S, H, D):
        return _reference_fallback(qw, kw, token_index, thread_id)

    qn_blocks = sorted({i for i in range(NB) if any(regimes[i][j] == "np" for j in range(NB))})
    kn_blocks = sorted({j for j in range(NB) if any(regimes[i][j] == "pn" for i in range(NB))})
    if not qn_blocks:
        qn_blocks = [0]
    if not kn_blocks:
        kn_blocks = [0]

    rots = _host_rotations(qw, kw, token_index)
    (qp_u, qp_v), (qn_u, qn_v), (kp_u, kp_v), (kn_u, kn_v) = rots
    all_blocks = list(range(NB))
    qp_a = _to_device_layout(qp_u, qp_v, all_blocks)  # (B,H,2,HALF,S)
    qn_a = _to_device_layout(qn_u, qn_v, qn_blocks)
    kp_a = _to_device_layout(kp_u, kp_v, all_blocks)
    kn_a = _to_device_layout(kn_u, kn_v, kn_blocks)

    key = (tuple(tuple(r) for r in regimes), tuple(qn_blocks), tuple(kn_blocks))
    if key not in _prog_cache:
        _prog_cache[key] = _build_program(regimes, qn_blocks, kn_blocks)
    nc = _prog_cache[key]

    from concourse.bass_utils import run_bass_kernel_spmd

    in_maps = [
        {"qp": qp_a[b], "qn": qn_a[b], "kp": kp_a[b], "kn": kn_a[b]}
        for b in range(B)
    ]
    trace = bool(int(os.environ.get("KERNEL_TRACE", "0")))
    res = run_bass_kernel_spmd(
        nc,
        in_maps,
        core_ids=list(range(N_CORES)),
        trace=trace,
    )
    if res.exec_time_ns is not None:
        print(f"HW exec time: {res.exec_time_ns} ns")
    if res.instructions_and_trace is not None:
        print(f"trace: {res.instructions_and_trace[1]}")

    out = np.stack([res.results[b]["out"] for b in range(B)], axis=0)
    return out.astype(np.float32)


# revision 6
# speedup vs baseline: 1.1266x; 1.1266x over previous
"""Trainium2 Bass kernel for nn_BertWordPair (ragged RoPE pair scores).

Strategy
--------
Inputs: qw, kw (B=8, S=768, H=4, D=256) fp32; token_index, thread_id (S,) int32.
Output: (B, S, S, H) fp32 where each (row-block, col-block) pair of the 6x128
thread-block grid uses one of three RoPE sign regimes:
    pp: rope(q,+pos) . rope(k,+pos)
    np: rope(q,-pos) . rope(k,+pos)   (0 < ti_r < ti_c)
    pn: rope(q,+pos) . rope(k,-pos)   (ti_c > 0, ti_r > ti_c)

Host side precomputes the four rotated variants (q+, q-, k+, k-) in a
de-interleaved (pair-index, token) layout, casts to fp16, and shards batch
across the 8 cores (1 dialogue per core). Device does only matmuls
(one 128x128x256 contraction per output block/head, fp16 in, fp32 PSUM),
PSUM->SBUF head-interleave copies (DVE/ACT alternating), and DMAs.
"""

import os

import numpy as np

ROPE_BASE = 10000.0
B, S, H, D = 8, 768, 4, 256
HALF = D // 2  # 128
BLK = 128
NB = S // BLK  # 6
N_CORES = 8

_prog_cache = {}


def _host_rotations(qw, kw, token_index):
    """Return u/v (even/odd) rotated variants, fp32.

    Shapes: (B, S, H, HALF) each for (qp_u, qp_v, qn_u, qn_v, kp_u, kp_v,
    kn_u, kn_v)."""
    inv_freq = np.power(
        np.float32(ROPE_BASE),
        (np.arange(HALF, dtype=np.float32) * np.float32(-2.0 / D)),
    )  # (HALF,)
    pos = token_index.astype(np.float32)  # (S,)
    theta = pos[:, None] * inv_freq[None, :]  # (S, HALF)
    cos = np.cos(theta)[None, :, None, :]  # (1,S,1,HALF)
    sin = np.sin(theta)[None, :, None, :]

    out = []
    for x in (qw, kw):
        u = x[..., 0::2]  # (B,S,H,HALF)
        v = x[..., 1::2]
        uc = u * cos
        vs = v * sin
        vc = v * cos
        us = u * sin
        # positive rotation
        out.append((uc - vs, vc + us))
        # negative rotation (sin -> -sin)
        out.append((uc + vs, vc - us))
    return out  # [(qp_u,qp_v),(qn_u,qn_v),(kp_u,kp_v),(kn_u,kn_v)]


def _to_device_layout(u, v, blocks):
    """(B,S,H,HALF) u/v -> (B, H, 2, HALF, T) fp16 for the given token blocks."""
    cols = np.concatenate([np.arange(b * BLK, (b + 1) * BLK) for b in blocks])
    u = u[:, cols]  # (B,T,H,HALF)
    v = v[:, cols]
    arr = np.stack([u, v], axis=2)  # (B,T,2,H,HALF)
    arr = np.transpose(arr, (0, 3, 2, 4, 1))  # (B,H,2,HALF,T)
    return np.ascontiguousarray(arr.astype(np.float16))


def _regime_map(thread_id):
    """Return (regimes, ok). regimes[i][j] in {'pp','np','pn'} per 128-block."""
    tid = np.asarray(thread_id)
    if tid.shape[0] != S:
        return None, False
    blocks = tid.reshape(NB, BLK)
    if not np.all(blocks == blocks[:, :1]):
        return None, False  # thread blocks not aligned to 128 grid
    tvals = blocks[:, 0]
    regimes = []
    for i in range(NB):
        row = []
        for j in range(NB):
            ti_r, ti_c = tvals[i], tvals[j]
            if ti_r > 0 and ti_r < ti_c:
                row.append("np")
            elif ti_c > 0 and ti_r > ti_c:
                row.append("pn")
            else:
                row.append("pp")
        regimes.append(row)
    return regimes, True


def _build_program(regimes, qn_blocks, kn_blocks):
    import concourse.bass as bass  # noqa: F401
    import concourse.tile as tile
    from concourse import bacc, mybir

    f16 = mybir.dt.float16
    f32 = mybir.dt.float32

    nqn = max(1, len(qn_blocks))
    nkn = max(1, len(kn_blocks))
    qn_pos = {b: idx for idx, b in enumerate(qn_blocks)}
    kn_pos = {b: idx for idx, b in enumerate(kn_blocks)}

    nc = bacc.Bacc(None, target_bir_lowering=False)
    qp_d = nc.dram_tensor("qp", [H, 2, HALF, S], f16, kind="ExternalInput")
    qn_d = nc.dram_tensor("qn", [H, 2, HALF, nqn * BLK], f16, kind="ExternalInput")
    kp_d = nc.dram_tensor("kp", [H, 2, HALF, S], f16, kind="ExternalInput")
    kn_d = nc.dram_tensor("kn", [H, 2, HALF, nkn * BLK], f16, kind="ExternalInput")
    out_d = nc.dram_tensor("out", [S, S, H], f32, kind="ExternalOutput")

    with tile.TileContext(nc) as tc:
        with (
            tc.tile_pool(name="inp", bufs=1) as inp,
            tc.tile_pool(name="psum", bufs=8, space="PSUM") as pp,
            tc.tile_pool(name="stage", bufs=3) as stp,
        ):
            # Load all inputs. Tiles are (128 partitions = pair index,
            # H*2*T tokens) fp16.
            qp_t = inp.tile([HALF, H * 2 * S], f16, tag="qp")
            qn_t = inp.tile([HALF, H * 2 * nqn * BLK], f16, tag="qn")
            kp_t = inp.tile([HALF, H * 2 * S], f16, tag="kp")
            kn_t = inp.tile([HALF, H * 2 * nkn * BLK], f16, tag="kn")
            # qp/kp first on the SP HWDGE ring, split by d-chunk half so row-0
            # matmuls (start-group, c=0) can begin after the first two DMAs;
            # qn/kn ride the ACT ring so they never queue ahead of output DMAs.
            for c in range(2):
                for t, d in ((qp_t, qp_d), (kp_t, kp_d)):
                    tlen = t.shape[1] // (H * 2)
                    nc.sync.dma_start(
                        t[:].rearrange("p (h c t) -> p h c t", h=H, c=2, t=tlen)[
                            :, :, c
                        ],
                        d[:].rearrange("h c p t -> p h c t")[:, :, c],
                    )
            for t, d in ((qn_t, qn_d), (kn_t, kn_d)):
                tlen = t.shape[1] // (H * 2)
                nc.sync.dma_start(
                    t[:].rearrange("p (h c t) -> p h c t", h=H, c=2, t=tlen),
                    d[:].rearrange("h c p t -> p h c t"),
                )

            def lhs_slice(variant, h, c, blk):
                if variant == "p":
                    return qp_t[:, (h * 2 + c) * S + blk * BLK :][:, :BLK]
                return qn_t[:, (h * 2 + c) * (nqn * BLK) + qn_pos[blk] * BLK :][:, :BLK]

            def rhs_slice(variant, h, c, blk):
                if variant == "p":
                    return kp_t[:, (h * 2 + c) * S + blk * BLK :][:, :BLK]
                return kn_t[:, (h * 2 + c) * (nkn * BLK) + kn_pos[blk] * BLK :][:, :BLK]

            copy_parity = 0
            for i in range(NB):
                stage = stp.tile([BLK, S * H], f32, tag="stage")
                for j in range(NB):
                    reg = regimes[i][j]
                    qv = "n" if reg == "np" else "p"
                    kv = "n" if reg == "pn" else "p"
                    # interleaved destination for block j: stage columns
                    # j*512 + n*4 + h
                    dst_blk = stage[:, j * (BLK * H) : (j + 1) * (BLK * H)]
                    dst_blk = dst_blk.rearrange("p (n h) -> p h n", h=H)
                    for h in range(H):
                        bank = pp.tile([BLK, BLK], f32, tag="bank")
                        for c in range(2):
                            nc.tensor.matmul(
                                bank[:],
                                lhs_slice(qv, h, c, i),
                                rhs_slice(kv, h, c, j),
                                start=(c == 0),
                                stop=(c == 1),
                            )
                        eng = nc.vector if copy_parity == 0 else nc.scalar
                        copy_parity ^= 1
                        if eng is nc.vector:
                            eng.tensor_copy(dst_blk[:, h], bank[:])
                        else:
                            eng.copy(dst_blk[:, h], bank[:])
                nc.sync.dma_start(
                    out_d[i * BLK : (i + 1) * BLK].rearrange("p n h -> p (n h)"),
                    stage[:],
                )
    nc.finalize()
    return nc


def _reference_fallback(qw, kw, token_index, thread_id):
    """Pure numpy fallback for unexpected block structure."""
    rots = _host_rotations(qw, kw, token_index)
    (qp_u, qp_v), (qn_u, qn_v), (kp_u, kp_v), (kn_u, kn_v) = rots

    def interleave(u, v):
        x = np.empty(u.shape[:-1] + (D,), dtype=np.float32)
        x[..., 0::2] = u
        x[..., 1::2] = v
        return x

    q_p = interleave(qp_u, qp_v)
    q_n = interleave(qn_u, qn_v)
    k_p = interleave(kp_u, kp_v)
    k_n = interleave(kn_u, kn_v)
    s_pp = np.einsum("bmhd,bnhd->bmnh", q_p, k_p)
    s_np = np.einsum("bmhd,bnhd->bmnh", q_n, k_p)
    s_pn = np.einsum("bmhd,bnhd->bmnh", q_p, k_n)
    ti_r = thread_id[:, None]
    ti_c = thread_id[None, :]
    sx = ((ti_r > 0) & (ti_r < ti_c))[None, :, :, None]
    sy = ((ti_c > 0) & (ti_r > ti_c))[None, :, :, None]
    return np.where(sx, s_np, np.where(sy, s_pn, s_pp)).astype(np.float32)


def kernel(qw, kw, token_index, thread_id):
    qw = np.asarray(qw, dtype=np.float32)
    kw = np.asarray(kw, dtype=np.float32)
    token_index = np.asarray(token_index)
    thread_id = np.asarray(thread_id)

    regimes, ok = _regime_map(thread_id)
    if not ok or qw.shape != (B, S, H, D):
        return _reference_fallback(qw, kw, token_index, thread_id)

    qn_blocks = sorted({i for i in range(NB) if any(regimes[i][j] == "np" for j in range(NB))})
    kn_blocks = sorted({j for j in range(NB) if any(regimes[i][j] == "pn" for i in range(NB))})
    if not qn_blocks:
        qn_blocks = [0]
    if not kn_blocks:
        kn_blocks = [0]

    rots = _host_rotations(qw, kw, token_index)
    (qp_u, qp_v), (qn_u, qn_v), (kp_u, kp_v), (kn_u, kn_v) = rots
    all_blocks = list(range(NB))
    qp_a = _to_device_layout(qp_u, qp_v, all_blocks)  # (B,H,2,HALF,S)
    qn_a = _to_device_layout(qn_u, qn_v, qn_blocks)
    kp_a = _to_device_layout(kp_u, kp_v, all_blocks)
    kn_a = _to_device_layout(kn_u, kn_v, kn_blocks)

    key = (tuple(tuple(r) for r in regimes), tuple(qn_blocks), tuple(kn_blocks))
    if key not in _prog_cache:
        _prog_cache[key] = _build_program(regimes, qn_blocks, kn_blocks)
    nc = _prog_cache[key]

    from concourse.bass_utils import run_bass_kernel_spmd

    in_maps = [
        {"qp": qp_a[b], "qn": qn_a[b], "kp": kp_a[b], "kn": kn_a[b]}
        for b in range(B)
    ]
    trace = bool(int(os.environ.get("KERNEL_TRACE", "0")))
    res = run_bass_kernel_spmd(
        nc,
        in_maps,
        core_ids=list(range(N_CORES)),
        trace=trace,
    )
    if res.exec_time_ns is not None:
        print(f"HW exec time: {res.exec_time_ns} ns")
    if res.instructions_and_trace is not None:
        print(f"trace: {res.instructions_and_trace[1]}")

    out = np.stack([res.results[b]["out"] for b in range(B)], axis=0)
    return out.astype(np.float32)


# revision 8
# speedup vs baseline: 1.1446x; 1.0159x over previous
"""Trainium2 Bass kernel for nn_BertWordPair (ragged RoPE pair scores).

Strategy
--------
Inputs: qw, kw (B=8, S=768, H=4, D=256) fp32; token_index, thread_id (S,) int32.
Output: (B, S, S, H) fp32 where each (row-block, col-block) pair of the 6x128
thread-block grid uses one of three RoPE sign regimes:
    pp: rope(q,+pos) . rope(k,+pos)
    np: rope(q,-pos) . rope(k,+pos)   (0 < ti_r < ti_c)
    pn: rope(q,+pos) . rope(k,-pos)   (ti_c > 0, ti_r > ti_c)

Host side precomputes the four rotated variants (q+, q-, k+, k-) in a
de-interleaved (pair-index, token) layout, casts to fp16, and shards batch
across the 8 cores (1 dialogue per core). Device does only matmuls
(one 128x128x256 contraction per output block/head, fp16 in, fp32 PSUM),
PSUM->SBUF head-interleave copies (DVE/ACT alternating), and DMAs.
"""

import os

import numpy as np

ROPE_BASE = 10000.0
B, S, H, D = 8, 768, 4, 256
HALF = D // 2  # 128
BLK = 128
NB = S // BLK  # 6
N_CORES = 8

_prog_cache = {}


def _host_rotations(qw, kw, token_index):
    """Return u/v (even/odd) rotated variants, fp32.

    Shapes: (B, S, H, HALF) each for (qp_u, qp_v, qn_u, qn_v, kp_u, kp_v,
    kn_u, kn_v)."""
    inv_freq = np.power(
        np.float32(ROPE_BASE),
        (np.arange(HALF, dtype=np.float32) * np.float32(-2.0 / D)),
    )  # (HALF,)
    pos = token_index.astype(np.float32)  # (S,)
    theta = pos[:, None] * inv_freq[None, :]  # (S, HALF)
    cos = np.cos(theta)[None, :, None, :]  # (1,S,1,HALF)
    sin = np.sin(theta)[None, :, None, :]

    out = []
    for x in (qw, kw):
        u = x[..., 0::2]  # (B,S,H,HALF)
        v = x[..., 1::2]
        uc = u * cos
        vs = v * sin
        vc = v * cos
        us = u * sin
        # positive rotation
        out.append((uc - vs, vc + us))
        # negative rotation (sin -> -sin)
        out.append((uc + vs, vc - us))
    return out  # [(qp_u,qp_v),(qn_u,qn_v),(kp_u,kp_v),(kn_u,kn_v)]


def _to_device_layout(u, v, blocks):
    """(B,S,H,HALF) u/v -> (B, H, 2, HALF, T) fp16 for the given token blocks."""
    cols = np.concatenate([np.arange(b * BLK, (b + 1) * BLK) for b in blocks])
    u = u[:, cols]  # (B,T,H,HALF)
    v = v[:, cols]
    arr = np.stack([u, v], axis=2)  # (B,T,2,H,HALF)
    arr = np.transpose(arr, (0, 3, 2, 4, 1))  # (B,H,2,HALF,T)
    return np.ascontiguousarray(arr.astype(np.float16))


def _regime_map(thread_id):
    """Return (regimes, ok). regimes[i][j] in {'pp','np','pn'} per 128-block."""
    tid = np.asarray(thread_id)
    if tid.shape[0] != S:
        return None, False
    blocks = tid.reshape(NB, BLK)
    if not np.all(blocks == blocks[:, :1]):
        return None, False  # thread blocks not aligned to 128 grid
    tvals = blocks[:, 0]
    regimes = []
    for i in range(NB):
        row = []
        for j in range(NB):
            ti_r, ti_c = tvals[i], tvals[j]
            if ti_r > 0 and ti_r < ti_c:
                row.append("np")
            elif ti_c > 0 and ti_r > ti_c:
                row.append("pn")
            else:
                row.append("pp")
        regimes.append(row)
    return regimes, True


def _build_program(regimes, qn_blocks, kn_blocks):
    import concourse.bass as bass  # noqa: F401
    import concourse.tile as tile
    from concourse import bacc, mybir

    f16 = mybir.dt.float16
    f32 = mybir.dt.float32

    nqn = max(1, len(qn_blocks))
    nkn = max(1, len(kn_blocks))
    qn_pos = {b: idx for idx, b in enumerate(qn_blocks)}
    kn_pos = {b: idx for idx, b in enumerate(kn_blocks)}

    nc = bacc.Bacc(None, target_bir_lowering=False)
    qp_d = nc.dram_tensor("qp", [H, 2, HALF, S], f16, kind="ExternalInput")
    qn_d = nc.dram_tensor("qn", [H, 2, HALF, nqn * BLK], f16, kind="ExternalInput")
    kp_d = nc.dram_tensor("kp", [H, 2, HALF, S], f16, kind="ExternalInput")
    kn_d = nc.dram_tensor("kn", [H, 2, HALF, nkn * BLK], f16, kind="ExternalInput")
    out_d = nc.dram_tensor("out", [S, S, H], f32, kind="ExternalOutput")

    with tile.TileContext(nc) as tc:
        with (
            tc.tile_pool(name="inp", bufs=1) as inp,
            tc.tile_pool(name="psum", bufs=8, space="PSUM") as pp,
            tc.tile_pool(name="stage", bufs=3) as stp,
        ):
            # Load all inputs. Tiles are (128 partitions = pair index,
            # H*2*T tokens) fp16.
            qp_t = inp.tile([HALF, H * 2 * S], f16, tag="qp")
            qn_t = inp.tile([HALF, H * 2 * nqn * BLK], f16, tag="qn")
            kp_t = inp.tile([HALF, H * 2 * S], f16, tag="kp")
            kn_t = inp.tile([HALF, H * 2 * nkn * BLK], f16, tag="kn")
            # qp/kp first on the SP HWDGE ring, split by d-chunk half so row-0
            # matmuls (start-group, c=0) can begin after the first two DMAs;
            # qn/kn ride the ACT ring so they never queue ahead of output DMAs.
            for c in range(2):
                for t, d in ((qp_t, qp_d), (kp_t, kp_d)):
                    tlen = t.shape[1] // (H * 2)
                    nc.sync.dma_start(
                        t[:].rearrange("p (h c t) -> p h c t", h=H, c=2, t=tlen)[
                            :, :, c
                        ],
                        d[:].rearrange("h c p t -> p h c t")[:, :, c],
                    )
            for t, d in ((qn_t, qn_d), (kn_t, kn_d)):
                tlen = t.shape[1] // (H * 2)
                nc.sync.dma_start(
                    t[:].rearrange("p (h c t) -> p h c t", h=H, c=2, t=tlen),
                    d[:].rearrange("h c p t -> p h c t"),
                )

            def lhs_slice(variant, h, c, blk):
                if variant == "p":
                    return qp_t[:, (h * 2 + c) * S + blk * BLK :][:, :BLK]
                return qn_t[:, (h * 2 + c) * (nqn * BLK) + qn_pos[blk] * BLK :][:, :BLK]

            def rhs_slice(variant, h, c, blk):
                if variant == "p":
                    return kp_t[:, (h * 2 + c) * S + blk * BLK :][:, :BLK]
                return kn_t[:, (h * 2 + c) * (nkn * BLK) + kn_pos[blk] * BLK :][:, :BLK]

            copy_parity = 0
            for i in range(NB):
                stage = stp.tile([BLK, S * H], f32, tag="stage")
                # One PSUM bank per (i, j) holds all 4 heads [h0|h1|h2|h3].
                # Only the first matmul into the bank uses start=True (the
                # bank-wide pending-zero clear); every element is written
                # exactly once per chunk, so per-element has_written handles
                # the rest. Emit all c=0 matmuls of the row before the c=1
                # matmuls so the PE FIFO isn't head-of-line blocked waiting
                # for the second-chunk input DMA.
                banks = {}
                for j in range(NB):
                    reg = regimes[i][j]
                    qv = "n" if reg == "np" else "p"
                    kv = "n" if reg == "pn" else "p"
                    bank = pp.tile([BLK, BLK * H], f32, tag="bank")
                    banks[j] = bank
                    for h in range(H):
                        nc.tensor.matmul(
                            bank[:, h * BLK : (h + 1) * BLK],
                            lhs_slice(qv, h, 0, i),
                            rhs_slice(kv, h, 0, j),
                            start=(h == 0),
                            stop=False,
                        )
                for j in range(NB):
                    reg = regimes[i][j]
                    qv = "n" if reg == "np" else "p"
                    kv = "n" if reg == "pn" else "p"
                    bank = banks[j]
                    for h in range(H):
                        nc.tensor.matmul(
                            bank[:, h * BLK : (h + 1) * BLK],
                            lhs_slice(qv, h, 1, i),
                            rhs_slice(kv, h, 1, j),
                            start=False,
                            stop=(h == H - 1),
                        )
                    # one head-interleaving evacuation copy per bank:
                    # bank (p, (h n)) -> stage (p, (n h)) at block j
                    dst_blk = stage[:, j * (BLK * H) : (j + 1) * (BLK * H)]
                    dst_blk = dst_blk.rearrange("p (n h) -> p h n", h=H)
                    src_blk = bank[:].rearrange("p (h n) -> p h n", n=BLK)
                    eng = nc.vector if copy_parity == 0 else nc.scalar
                    copy_parity ^= 1
                    if eng is nc.vector:
                        eng.tensor_copy(dst_blk, src_blk)
                    else:
                        eng.copy(dst_blk, src_blk)
                nc.sync.dma_start(
                    out_d[i * BLK : (i + 1) * BLK].rearrange("p n h -> p (n h)"),
                    stage[:],
                )
    nc.finalize()
    return nc


def _reference_fallback(qw, kw, token_index, thread_id):
    """Pure numpy fallback for unexpected block structure."""
    rots = _host_rotations(qw, kw, token_index)
    (qp_u, qp_v), (qn_u, qn_v), (kp_u, kp_v), (kn_u, kn_v) = rots

    def interleave(u, v):
        x = np.empty(u.shape[:-1] + (D,), dtype=np.float32)
        x[..., 0::2] = u
        x[..., 1::2] = v
        return x

    q_p = interleave(qp_u, qp_v)
    q_n = interleave(qn_u, qn_v)
    k_p = interleave(kp_u, kp_v)
    k_n = interleave(kn_u, kn_v)
    s_pp = np.einsum("bmhd,bnhd->bmnh", q_p, k_p)
    s_np = np.einsum("bmhd,bnhd->bmnh", q_n, k_p)
    s_pn = np.einsum("bmhd,bnhd->bmnh", q_p, k_n)
    ti_r = thread_id[:, None]
    ti_c = thread_id[None, :]
    sx = ((ti_r > 0) & (ti_r < ti_c))[None, :, :, None]
    sy = ((ti_c > 0) & (ti_r > ti_c))[None, :, :, None]
    return np.where(sx, s_np, np.where(sy, s_pn, s_pp)).astype(np.float32)


def kernel(qw, kw, token_index, thread_id):
    qw = np.asarray(qw, dtype=np.float32)
    kw = np.asarray(kw, dtype=np.float32)
    token_index = np.asarray(token_index)
    thread_id = np.asarray(thread_id)

    regimes, ok = _regime_map(thread_id)
    if not ok or qw.shape != (B, S, H, D):
        return _reference_fallback(qw, kw, token_index, thread_id)

    qn_blocks = sorted({i for i in range(NB) if any(regimes[i][j] == "np" for j in range(NB))})
    kn_blocks = sorted({j for j in range(NB) if any(regimes[i][j] == "pn" for i in range(NB))})
    if not qn_blocks:
        qn_blocks = [0]
    if not kn_blocks:
        kn_blocks = [0]

    rots = _host_rotations(qw, kw, token_index)
    (qp_u, qp_v), (qn_u, qn_v), (kp_u, kp_v), (kn_u, kn_v) = rots
    all_blocks = list(range(NB))
    qp_a = _to_device_layout(qp_u, qp_v, all_blocks)  # (B,H,2,HALF,S)
    qn_a = _to_device_layout(qn_u, qn_v, qn_blocks)
    kp_a = _to_device_layout(kp_u, kp_v, all_blocks)
    kn_a = _to_device_layout(kn_u, kn_v, kn_blocks)

    key = (tuple(tuple(r) for r in regimes), tuple(qn_blocks), tuple(kn_blocks))
    if key not in _prog_cache:
        _prog_cache[key] = _build_program(regimes, qn_blocks, kn_blocks)
    nc = _prog_cache[key]

    from concourse.bass_utils import run_bass_kernel_spmd

    in_maps = [
        {"qp": qp_a[b], "qn": qn_a[b], "kp": kp_a[b], "kn": kn_a[b]}
        for b in range(B)
    ]
    trace = bool(int(os.environ.get("KERNEL_TRACE", "0")))
    res = run_bass_kernel_spmd(
        nc,
        in_maps,
        core_ids=list(range(N_CORES)),
        trace=trace,
    )
    if res.exec_time_ns is not None:
        print(f"HW exec time: {res.exec_time_ns} ns")
    if res.instructions_and_trace is not None:
        print(f"trace: {res.instructions_and_trace[1]}")

    out = np.stack([res.results[b]["out"] for b in range(B)], axis=0)
    return out.astype(np.float32)
